# revision 25
# baseline (speedup 1.0000x reference)
"""Trainium2 Bass kernel for the Antecedent (fuzzy firing strength) problem.

fir[s, r] = exp(sum_d logmv[s, fs_ind[r, d], d])
with logmv[s, f, d] = -(x[s,d] - c[f,d])^2 / (2 * spread[f,d]^2)

Key idea: the gather+sum over d is a matmul with K = num_fs*in_dim = 32:
    fir[s, r] = exp( sum_k onehot[k, r] * logmvT[k, s] ),
    onehot[f*8+d, r] = 1 iff fs_ind[r, d] == f   (0/1 layout encoding, host-built)
    logmvT[f*8+d, s] = -(x[s,d]-c[f,d])^2 / (2*spread[f,d]^2)   (device-computed)

Sharding: rules split across the 8 cores (8192 rules each); samples replicated.
Each core: 64 matmuls [K=32, M=128 samples, N=512 rules] -> PSUM, ScalarE Exp
PSUM->SBUF, 4MB DMAs to its [512, 8192] output slice. Output write (16MB/core)
is the roofline term (~358 GB/s HBM per core).
"""

import sys

if "/opt/trn_rl_repo" not in sys.path:
    sys.path.insert(0, "/opt/trn_rl_repo")

import ml_dtypes
import numpy as np

import concourse.bacc as bacc
import concourse.bass as bass
import concourse.mybir as mybir
import concourse.tile as tile
from concourse.bass_utils import run_bass_kernel_spmd

NUM_SAM = 512
IN_DIM = 8
NUM_FS = 4
NUM_RULE = 65536
K = NUM_FS * IN_DIM  # 32 contraction size
N_CORES = 8
RPC = NUM_RULE // N_CORES  # 8192 rules per core

F32 = mybir.dt.float32
F32R = mybir.dt.float32r  # fp32 bits, 1 cycle/row PE stream rate (vs 4 for fp32)
BF16 = mybir.dt.bfloat16
OUT_DT = BF16  # fir values are exp(<=0) in (0,1]; bf16 keeps rel err ~1e-3

# loop tiling (per core)
N_SG = NUM_SAM // 128          # 4 sample groups of 128 (partition dim)
N_MM = 4                       # matmuls per exp group (512 rules)
MM_N = 512                     # moving free dim per matmul
EXP_N = N_MM * MM_N            # 2048 rules per exp + output DMA group
N_GRP = RPC // EXP_N           # 4 groups per sample group


def build_nc():
    nc = bacc.Bacc("TRN2", target_bir_lowering=False, debug=False, num_devices=N_CORES)

    oh_ext = nc.dram_tensor("onehot", [K, RPC], BF16, kind="ExternalInput")
    # xcs: cols 0..NUM_SAM-1 = x[s,d] tiled over f; col NUM_SAM = center,
    # col NUM_SAM+1 = spread (single input DMA for the whole prologue)
    xcs_ext = nc.dram_tensor("xcs", [K, NUM_SAM + 2], F32, kind="ExternalInput")
    out_ext = nc.dram_tensor("out", [NUM_SAM, RPC], OUT_DT, kind="ExternalOutput")

    with tile.TileContext(nc) as tc:
        with (
            tc.tile_pool(name="const", bufs=1) as cpool,
            tc.tile_pool(name="stage", bufs=3) as spool,
            tc.tile_pool(name="psum", bufs=2, space="PSUM") as ppool,
        ):
            # ---- prologue: tiny inputs + membership table ----
            xcs = cpool.tile([K, NUM_SAM + 2], F32)
            nc.sync.dma_start(out=xcs[:], in_=xcs_ext[:])
            xt32 = xcs[:, 0:NUM_SAM]
            cvec = xcs[:, NUM_SAM : NUM_SAM + 1]
            svec = xcs[:, NUM_SAM + 1 : NUM_SAM + 2]

            # one-hot rule encoding [K, RPC] with entries -1 (carries the
            # minus sign of the gaussian exponent); issued on the Scalar
            # HWDGE queue so it doesn't serialize behind Sync's const DMA
            oh = cpool.tile([K, RPC], BF16)
            n_chunks = 4
            csz = RPC // n_chunks
            for c in range(n_chunks):
                nc.scalar.dma_start(
                    out=oh[:, c * csz : (c + 1) * csz],
                    in_=oh_ext[:, c * csz : (c + 1) * csz],
                )

            # d2[k, s] = (x - c) / (s * sqrt(2)); lhsT = d2^2 (bf16).
            # The exponent's minus sign lives in the -1 one-hot entries.
            rsv = cpool.tile([K, 1], F32)
            tvec = cpool.tile([K, 1], F32)
            nc.vector.reciprocal(rsv[:], svec)
            nc.vector.tensor_scalar_mul(tvec[:], rsv[:], 0.7071067811865476)
            d2 = cpool.tile([K, NUM_SAM], F32)
            lhs_b = cpool.tile([K, NUM_SAM], BF16)
            nc.vector.tensor_scalar(
                d2[:], xt32, cvec, tvec[:],
                mybir.AluOpType.subtract, mybir.AluOpType.mult,
            )
            nc.vector.tensor_mul(lhs_b[:], d2[:], d2[:])

            # ---- main loop ----
            Exp = mybir.ActivationFunctionType.Exp
            for sg in range(N_SG):
                lhsT = lhs_b[:, sg * 128 : (sg + 1) * 128]  # [32, 128]
                for g in range(N_GRP):
                    stg = spool.tile([128, EXP_N], OUT_DT)
                    ps = ppool.tile([128, EXP_N], F32)
                    for j in range(N_MM):
                        rt = g * N_MM + j
                        nc.tensor.matmul(
                            ps[:, j * MM_N : (j + 1) * MM_N],
                            lhsT,
                            oh[:, rt * MM_N : (rt + 1) * MM_N],
                            start=True,
                            stop=True,
                        )
                    first = sg == 0 and g == 0
                    last = sg == N_SG - 1 and g == N_GRP - 1
                    out_slice = out_ext[
                        sg * 128 : (sg + 1) * 128, g * EXP_N : (g + 1) * EXP_N
                    ]
                    if first:
                        # 512-col exps so ACT starts right after matmul 1
                        for j in range(N_MM):
                            sl = slice(j * MM_N, (j + 1) * MM_N)
                            nc.scalar.activation(stg[:, sl], ps[:, sl], Exp)
                        nc.sync.dma_start(out=out_slice, in_=stg[:])
                    elif last:
                        # 512-col exp+DMA pairs to shrink the drain tail
                        for j in range(N_MM):
                            sl = slice(j * MM_N, (j + 1) * MM_N)
                            nc.scalar.activation(stg[:, sl], ps[:, sl], Exp)
                            nc.sync.dma_start(
                                out=out_slice[:, sl], in_=stg[:, sl]
                            )
                    else:
                        nc.scalar.activation(stg[:], ps[:], Exp)
                        nc.sync.dma_start(out=out_slice, in_=stg[:])

    nc.compile()
    return nc


def _prep_in_maps(model_input, center, spread, fs_ind):
    model_input = np.ascontiguousarray(model_input, dtype=np.float32)
    center = np.ascontiguousarray(center, dtype=np.float32)
    spread = np.ascontiguousarray(spread, dtype=np.float32)
    fs = np.clip(np.asarray(fs_ind), 0, NUM_FS - 1).astype(np.int64)

    # one-hot: oh[f*IN_DIM + d, r] = -1 iff fs_ind[r, d] == f (sign of the
    # gaussian exponent is folded in here)
    oh = np.zeros((K, NUM_RULE), dtype=ml_dtypes.bfloat16)
    r = np.arange(NUM_RULE)
    for d in range(IN_DIM):
        oh[fs[:, d] * IN_DIM + d, r] = -1.0

    # xcs: x transposed/tiled over f, plus center and spread columns
    xcs = np.empty((K, NUM_SAM + 2), dtype=np.float32)
    xcs[:, :NUM_SAM] = np.tile(model_input.T, (NUM_FS, 1))
    xcs[:, NUM_SAM] = center.reshape(K)
    xcs[:, NUM_SAM + 1] = spread.reshape(K)

    return [
        {
            "onehot": np.ascontiguousarray(oh[:, i * RPC : (i + 1) * RPC]),
            "xcs": xcs,
        }
        for i in range(N_CORES)
    ]


def _run(inputs, trace=False, **spmd_kwargs):
    in_maps = _prep_in_maps(
        inputs["model_input"], inputs["center"], inputs["spread"], inputs["fs_ind"]
    )
    nc = build_nc()
    res = run_bass_kernel_spmd(
        nc, in_maps, core_ids=list(range(N_CORES)), trace=trace, **spmd_kwargs
    )
    out = np.concatenate(
        [res.results[i]["out"].astype(np.float32) for i in range(N_CORES)], axis=1
    )
    return out, res


def kernel(model_input, center, spread, fs_ind):
    out, _ = _run(
        {
            "model_input": model_input,
            "center": center,
            "spread": spread,
            "fs_ind": fs_ind,
        }
    )
    return out


# revision 26
# speedup vs baseline: 1.0212x; 1.0212x over previous
"""Trainium2 Bass kernel for the Antecedent (fuzzy firing strength) problem.

fir[s, r] = exp(sum_d logmv[s, fs_ind[r, d], d])
with logmv[s, f, d] = -(x[s,d] - c[f,d])^2 / (2 * spread[f,d]^2)

Key idea: the gather+sum over d is a matmul with K = num_fs*in_dim = 32:
    fir[s, r] = exp( sum_k onehot[k, r] * logmvT[k, s] ),
    onehot[f*8+d, r] = 1 iff fs_ind[r, d] == f   (0/1 layout encoding, host-built)
    logmvT[f*8+d, s] = -(x[s,d]-c[f,d])^2 / (2*spread[f,d]^2)   (device-computed)

Sharding: rules split across the 8 cores (8192 rules each); samples replicated.
Each core: 64 matmuls [K=32, M=128 samples, N=512 rules] -> PSUM, ScalarE Exp
PSUM->SBUF, 4MB DMAs to its [512, 8192] output slice. Output write (16MB/core)
is the roofline term (~358 GB/s HBM per core).
"""

import sys

if "/opt/trn_rl_repo" not in sys.path:
    sys.path.insert(0, "/opt/trn_rl_repo")

import ml_dtypes
import numpy as np

import concourse.bacc as bacc
import concourse.bass as bass
import concourse.mybir as mybir
import concourse.tile as tile
from concourse.bass_utils import run_bass_kernel_spmd

NUM_SAM = 512
IN_DIM = 8
NUM_FS = 4
NUM_RULE = 65536
K = NUM_FS * IN_DIM  # 32 contraction size
N_CORES = 8
RPC = NUM_RULE // N_CORES  # 8192 rules per core

F32 = mybir.dt.float32
F32R = mybir.dt.float32r  # fp32 bits, 1 cycle/row PE stream rate (vs 4 for fp32)
BF16 = mybir.dt.bfloat16
OUT_DT = BF16  # fir values are exp(<=0) in (0,1]; bf16 keeps rel err ~1e-3

# loop tiling (per core)
N_SG = NUM_SAM // 128          # 4 sample groups of 128 (partition dim)
N_MM = 4                       # matmuls per exp group (512 rules)
MM_N = 512                     # moving free dim per matmul
EXP_N = N_MM * MM_N            # 2048 rules per exp + output DMA group
N_GRP = RPC // EXP_N           # 4 groups per sample group


def build_nc():
    nc = bacc.Bacc("TRN2", target_bir_lowering=False, debug=False, num_devices=N_CORES)

    oh_ext = nc.dram_tensor("onehot", [K, RPC], BF16, kind="ExternalInput")
    # xcs: cols 0..NUM_SAM-1 = x[s,d] tiled over f; col NUM_SAM = center,
    # col NUM_SAM+1 = spread (single input DMA for the whole prologue)
    xcs_ext = nc.dram_tensor("xcs", [K, NUM_SAM + 2], F32, kind="ExternalInput")
    out_ext = nc.dram_tensor("out", [NUM_SAM, RPC], OUT_DT, kind="ExternalOutput")

    with tile.TileContext(nc) as tc:
        with (
            tc.tile_pool(name="const", bufs=1) as cpool,
            tc.tile_pool(name="stage", bufs=3) as spool,
            tc.tile_pool(name="psum", bufs=2, space="PSUM") as ppool,
        ):
            # ---- prologue: tiny inputs + membership table ----
            xcs = cpool.tile([K, NUM_SAM + 2], F32)
            nc.sync.dma_start(out=xcs[:], in_=xcs_ext[:])
            xt32 = xcs[:, 0:NUM_SAM]
            cvec = xcs[:, NUM_SAM : NUM_SAM + 1]
            svec = xcs[:, NUM_SAM + 1 : NUM_SAM + 2]

            # one-hot rule encoding [K, RPC] with entries -1 (carries the
            # minus sign of the gaussian exponent); issued on the Scalar
            # HWDGE queue so it doesn't serialize behind Sync's const DMA
            oh = cpool.tile([K, RPC], BF16)
            n_chunks = 4
            csz = RPC // n_chunks
            for c in range(n_chunks):
                nc.scalar.dma_start(
                    out=oh[:, c * csz : (c + 1) * csz],
                    in_=oh_ext[:, c * csz : (c + 1) * csz],
                )

            # d2[k, s] = (x - c) / (s * sqrt(2)); lhsT = d2^2 (bf16).
            # The exponent's minus sign lives in the -1 one-hot entries.
            rsv = cpool.tile([K, 1], F32)
            tvec = cpool.tile([K, 1], F32)
            nc.vector.reciprocal(rsv[:], svec)
            nc.vector.tensor_scalar_mul(tvec[:], rsv[:], 0.7071067811865476)
            d2 = cpool.tile([K, NUM_SAM], F32)
            lhs_b = cpool.tile([K, NUM_SAM], BF16)
            nc.vector.tensor_scalar(
                d2[:], xt32, cvec, tvec[:],
                mybir.AluOpType.subtract, mybir.AluOpType.mult,
            )
            nc.vector.tensor_mul(lhs_b[:], d2[:], d2[:])

            # ---- main loop ----
            Exp = mybir.ActivationFunctionType.Exp
            for sg in range(N_SG):
                lhsT = lhs_b[:, sg * 128 : (sg + 1) * 128]  # [32, 128]
                for g in range(N_GRP):
                    stg = spool.tile([128, EXP_N], OUT_DT)
                    ps = ppool.tile([128, EXP_N], F32)
                    for j in range(N_MM):
                        rt = g * N_MM + j
                        nc.tensor.matmul(
                            ps[:, j * MM_N : (j + 1) * MM_N],
                            lhsT,
                            oh[:, rt * MM_N : (rt + 1) * MM_N],
                            start=True,
                            stop=True,
                        )
                    out_slice = out_ext[
                        sg * 128 : (sg + 1) * 128, g * EXP_N : (g + 1) * EXP_N
                    ]
                    if sg == 0 and g == 0:
                        # split the first exp so ACT starts after 1 matmul
                        nc.scalar.activation(stg[:, :MM_N], ps[:, :MM_N], Exp)
                        nc.scalar.activation(stg[:, MM_N:], ps[:, MM_N:], Exp)
                    else:
                        nc.scalar.activation(stg[:], ps[:], Exp)
                    nc.sync.dma_start(out=out_slice, in_=stg[:])

    nc.compile()
    return nc


def _prep_in_maps(model_input, center, spread, fs_ind):
    model_input = np.ascontiguousarray(model_input, dtype=np.float32)
    center = np.ascontiguousarray(center, dtype=np.float32)
    spread = np.ascontiguousarray(spread, dtype=np.float32)
    fs = np.clip(np.asarray(fs_ind), 0, NUM_FS - 1).astype(np.int64)

    # one-hot: oh[f*IN_DIM + d, r] = -1 iff fs_ind[r, d] == f (sign of the
    # gaussian exponent is folded in here)
    oh = np.zeros((K, NUM_RULE), dtype=ml_dtypes.bfloat16)
    r = np.arange(NUM_RULE)
    for d in range(IN_DIM):
        oh[fs[:, d] * IN_DIM + d, r] = -1.0

    # xcs: x transposed/tiled over f, plus center and spread columns
    xcs = np.empty((K, NUM_SAM + 2), dtype=np.float32)
    xcs[:, :NUM_SAM] = np.tile(model_input.T, (NUM_FS, 1))
    xcs[:, NUM_SAM] = center.reshape(K)
    xcs[:, NUM_SAM + 1] = spread.reshape(K)

    return [
        {
            "onehot": np.ascontiguousarray(oh[:, i * RPC : (i + 1) * RPC]),
            "xcs": xcs,
        }
        for i in range(N_CORES)
    ]


def _run(inputs, trace=False, **spmd_kwargs):
    in_maps = _prep_in_maps(
        inputs["model_input"], inputs["center"], inputs["spread"], inputs["fs_ind"]
    )
    nc = build_nc()
    res = run_bass_kernel_spmd(
        nc, in_maps, core_ids=list(range(N_CORES)), trace=trace, **spmd_kwargs
    )
    out = np.concatenate(
        [res.results[i]["out"].astype(np.float32) for i in range(N_CORES)], axis=1
    )
    return out, res


def kernel(model_input, center, spread, fs_ind):
    out, _ = _run(
        {
            "model_input": model_input,
            "center": center,
            "spread": spread,
            "fs_ind": fs_ind,
        }
    )
    return out


# revision 28
# speedup vs baseline: 1.0318x; 1.0104x over previous
"""Trainium2 Bass kernel for the Antecedent (fuzzy firing strength) problem.

fir[s, r] = exp(sum_d logmv[s, fs_ind[r, d], d])
with logmv[s, f, d] = -(x[s,d] - c[f,d])^2 / (2 * spread[f,d]^2)

Key idea: the gather+sum over d is a matmul with K = num_fs*in_dim = 32:
    fir[s, r] = exp( sum_k onehot[k, r] * logmvT[k, s] ),
    onehot[f*8+d, r] = 1 iff fs_ind[r, d] == f   (0/1 layout encoding, host-built)
    logmvT[f*8+d, s] = -(x[s,d]-c[f,d])^2 / (2*spread[f,d]^2)   (device-computed)

Sharding: rules split across the 8 cores (8192 rules each); samples replicated.
Each core: 64 matmuls [K=32, M=128 samples, N=512 rules] -> PSUM, ScalarE Exp
PSUM->SBUF, 4MB DMAs to its [512, 8192] output slice. Output write (16MB/core)
is the roofline term (~358 GB/s HBM per core).
"""

import sys

if "/opt/trn_rl_repo" not in sys.path:
    sys.path.insert(0, "/opt/trn_rl_repo")

import ml_dtypes
import numpy as np

import concourse.bacc as bacc
import concourse.bass as bass
import concourse.mybir as mybir
import concourse.tile as tile
from concourse.bass_utils import run_bass_kernel_spmd

NUM_SAM = 512
IN_DIM = 8
NUM_FS = 4
NUM_RULE = 65536
K = NUM_FS * IN_DIM  # 32 contraction size
N_CORES = 8
RPC = NUM_RULE // N_CORES  # 8192 rules per core

F32 = mybir.dt.float32
F32R = mybir.dt.float32r  # fp32 bits, 1 cycle/row PE stream rate (vs 4 for fp32)
BF16 = mybir.dt.bfloat16
OUT_DT = BF16  # fir values are exp(<=0) in (0,1]; bf16 keeps rel err ~1e-3

# loop tiling (per core)
N_SG = NUM_SAM // 128          # 4 sample groups of 128 (partition dim)
N_MM = 4                       # matmuls per exp group (512 rules)
MM_N = 512                     # moving free dim per matmul
EXP_N = N_MM * MM_N            # 2048 rules per exp + output DMA group
N_GRP = RPC // EXP_N           # 4 groups per sample group


def build_nc():
    nc = bacc.Bacc("TRN2", target_bir_lowering=False, debug=False, num_devices=N_CORES)

    oh_ext = nc.dram_tensor("onehot", [K, RPC], BF16, kind="ExternalInput")
    # xcs: cols 0..NUM_SAM-1 = x[s,d] tiled over f; col NUM_SAM = center,
    # col NUM_SAM+1 = spread (single input DMA for the whole prologue)
    xcs_ext = nc.dram_tensor("xcs", [K, NUM_SAM + 2], F32, kind="ExternalInput")
    out_ext = nc.dram_tensor("out", [NUM_SAM, RPC], OUT_DT, kind="ExternalOutput")

    with tile.TileContext(nc) as tc:
        with (
            tc.tile_pool(name="const", bufs=1) as cpool,
            tc.tile_pool(name="stage", bufs=3) as spool,
            tc.tile_pool(name="psum", bufs=2, space="PSUM") as ppool,
        ):
            # ---- prologue: tiny inputs + membership table ----
            xcs = cpool.tile([K, NUM_SAM + 2], F32)
            nc.sync.dma_start(out=xcs[:], in_=xcs_ext[:])
            xt32 = xcs[:, 0:NUM_SAM]
            cvec = xcs[:, NUM_SAM : NUM_SAM + 1]
            svec = xcs[:, NUM_SAM + 1 : NUM_SAM + 2]

            # one-hot rule encoding [K, RPC] with entries -1 (carries the
            # minus sign of the gaussian exponent); issued on the Scalar
            # HWDGE queue so it doesn't serialize behind Sync's const DMA
            oh = cpool.tile([K, RPC], BF16)
            c0 = 0
            for csz in (MM_N, 2560, 2560, 2560):  # small first chunk: the
                nc.scalar.dma_start(             # first matmul starts sooner
                    out=oh[:, c0 : c0 + csz],
                    in_=oh_ext[:, c0 : c0 + csz],
                )
                c0 += csz

            # d2[k, s] = (x - c) / (s * sqrt(2)); lhsT = d2^2 (bf16).
            # The exponent's minus sign lives in the -1 one-hot entries.
            rsv = cpool.tile([K, 1], F32)
            tvec = cpool.tile([K, 1], F32)
            nc.vector.reciprocal(rsv[:], svec)
            nc.vector.tensor_scalar_mul(tvec[:], rsv[:], 0.7071067811865476)
            d2 = cpool.tile([K, NUM_SAM], F32)
            lhs_b = cpool.tile([K, NUM_SAM], BF16)
            nc.vector.tensor_scalar(
                d2[:], xt32, cvec, tvec[:],
                mybir.AluOpType.subtract, mybir.AluOpType.mult,
            )
            nc.vector.tensor_mul(lhs_b[:], d2[:], d2[:])

            # ---- main loop ----
            Exp = mybir.ActivationFunctionType.Exp
            for sg in range(N_SG):
                lhsT = lhs_b[:, sg * 128 : (sg + 1) * 128]  # [32, 128]
                for g in range(N_GRP):
                    stg = spool.tile([128, EXP_N], OUT_DT)
                    ps = ppool.tile([128, EXP_N], F32)
                    for j in range(N_MM):
                        rt = g * N_MM + j
                        nc.tensor.matmul(
                            ps[:, j * MM_N : (j + 1) * MM_N],
                            lhsT,
                            oh[:, rt * MM_N : (rt + 1) * MM_N],
                            start=True,
                            stop=True,
                        )
                    out_slice = out_ext[
                        sg * 128 : (sg + 1) * 128, g * EXP_N : (g + 1) * EXP_N
                    ]
                    if sg == 0 and g == 0:
                        # staircase exps so ACT starts right after matmul 1
                        nc.scalar.activation(stg[:, :512], ps[:, :512], Exp)
                        nc.scalar.activation(stg[:, 512:1024], ps[:, 512:1024], Exp)
                        nc.scalar.activation(stg[:, 1024:], ps[:, 1024:], Exp)
                        nc.sync.dma_start(out=out_slice, in_=stg[:])
                    elif sg == N_SG - 1 and g == N_GRP - 1:
                        nc.scalar.activation(stg[:], ps[:], Exp)
                        # two half DMAs run on disjoint queues -> shorter tail
                        h = EXP_N // 2
                        nc.sync.dma_start(out=out_slice[:, :h], in_=stg[:, :h])
                        nc.sync.dma_start(out=out_slice[:, h:], in_=stg[:, h:])
                    else:
                        nc.scalar.activation(stg[:], ps[:], Exp)
                        nc.sync.dma_start(out=out_slice, in_=stg[:])

    nc.compile()
    return nc


def _prep_in_maps(model_input, center, spread, fs_ind):
    model_input = np.ascontiguousarray(model_input, dtype=np.float32)
    center = np.ascontiguousarray(center, dtype=np.float32)
    spread = np.ascontiguousarray(spread, dtype=np.float32)
    fs = np.clip(np.asarray(fs_ind), 0, NUM_FS - 1).astype(np.int64)

    # one-hot: oh[f*IN_DIM + d, r] = -1 iff fs_ind[r, d] == f (sign of the
    # gaussian exponent is folded in here)
    oh = np.zeros((K, NUM_RULE), dtype=ml_dtypes.bfloat16)
    r = np.arange(NUM_RULE)
    for d in range(IN_DIM):
        oh[fs[:, d] * IN_DIM + d, r] = -1.0

    # xcs: x transposed/tiled over f, plus center and spread columns
    xcs = np.empty((K, NUM_SAM + 2), dtype=np.float32)
    xcs[:, :NUM_SAM] = np.tile(model_input.T, (NUM_FS, 1))
    xcs[:, NUM_SAM] = center.reshape(K)
    xcs[:, NUM_SAM + 1] = spread.reshape(K)

    return [
        {
            "onehot": np.ascontiguousarray(oh[:, i * RPC : (i + 1) * RPC]),
            "xcs": xcs,
        }
        for i in range(N_CORES)
    ]


def _run(inputs, trace=False, **spmd_kwargs):
    in_maps = _prep_in_maps(
        inputs["model_input"], inputs["center"], inputs["spread"], inputs["fs_ind"]
    )
    nc = build_nc()
    res = run_bass_kernel_spmd(
        nc, in_maps, core_ids=list(range(N_CORES)), trace=trace, **spmd_kwargs
    )
    out = np.concatenate(
        [res.results[i]["out"].astype(np.float32) for i in range(N_CORES)], axis=1
    )
    return out, res


def kernel(model_input, center, spread, fs_ind):
    out, _ = _run(
        {
            "model_input": model_input,
            "center": center,
            "spread": spread,
            "fs_ind": fs_ind,
        }
    )
    return out
